# revision 8
# baseline (speedup 1.0000x reference)
"""MoE (8 experts, top-2) Trainium2 kernel.

Strategy (expert-parallel, per sharding hint):
  - Host: router (softmax + top-2 + renorm)  [0.1% of FLOPs], then
    all-to-all dispatch done host-side: gather each expert's tokens into a
    fixed-capacity buffer.
  - Device (8 cores, 1 expert each): Y_e = GELU(X_e @ W1[e] + b1[e]) @ W2[e] + b2[e]
    Matmuls run in fp32r (tf32) on the PE array: full 1-cycle/row rate.
  - Host: weighted combine (the return all-to-all) back to [B, L, D].

Device loop per core (capacity C tokens, chunks of 768):
  mm1: psum1[f128, tok] = sum_dt W1t[dt,ft].T @ Xt[dt, tok]   (f32r)
  h[ft] = Gelu(psum1 + b1[ft])  -> SBUF f32r (resident for whole chunk)
  mm2: psum2[d128, tok] = sum_ft W2t[ft,dt2].T @ h[ft, tok]   (f32r)
  y = psum2 + b2 -> DRAM
"""

import numpy as np

import concourse.bacc as bacc
import concourse.mybir as mybir
import concourse.tile as tile
from concourse.bass import ds, ts
from concourse.bass_utils import run_bass_kernel_spmd

P = 128
D_MODEL = 1024
D_FF = 4096
NUM_EXPERTS = 8
TOP_K = 2
NDT = D_MODEL // P   # 8  d-tiles
NFT = D_FF // P      # 32 f-tiles
CAP_DEFAULT = 2304   # tokens-per-expert capacity (multiple of TCH)
TCH = 768            # token chunk resident in SBUF
SUB = 384            # matmul moving free dim (>=256 keeps fp32r at full rate)

f32 = mybir.dt.float32
f32r = mybir.dt.float32r

_BUILT = {}


def _tf32(a: np.ndarray) -> np.ndarray:
    """Round-to-nearest-even fp32 -> tf32 (19 low mantissa bits -> 10)."""
    u = np.ascontiguousarray(a, dtype=np.float32).view(np.uint32)
    u = (u + np.uint32(0xFFF) + ((u >> np.uint32(13)) & np.uint32(1))) & np.uint32(
        0xFFFFE000
    )
    return u.view(np.float32)


def _build(cap: int):
    """Build the per-core expert-MLP Bass module for a given capacity."""
    nc = bacc.Bacc(None, target_bir_lowering=False)

    xt = nc.declare_dram_parameter("xt", [P, NDT, cap], f32r, isOutput=False)
    w1 = nc.declare_dram_parameter("w1", [P, NFT, NDT, P], f32r, isOutput=False)
    w2 = nc.declare_dram_parameter("w2", [P, NDT, NFT, P], f32r, isOutput=False)
    b1 = nc.declare_dram_parameter("b1", [P, NFT], f32, isOutput=False)
    b2 = nc.declare_dram_parameter("b2", [P, NDT], f32, isOutput=False)
    y = nc.declare_dram_parameter("y", [P, NDT, cap], f32, isOutput=True)

    n_tch = cap // TCH
    nsub = TCH // SUB

    with tile.TileContext(nc) as tc:
        with (
            tc.tile_pool(name="const", bufs=1) as const_pool,
            tc.tile_pool(name="xt", bufs=1) as xt_pool,
            tc.tile_pool(name="w1", bufs=2) as w1_pool,
            tc.tile_pool(name="w2", bufs=2) as w2_pool,
            tc.tile_pool(name="h", bufs=1) as h_pool,
            tc.tile_pool(name="yo", bufs=2) as y_pool,
            tc.tile_pool(name="ps1", bufs=2, space="PSUM") as ps1_pool,
            tc.tile_pool(name="ps2", bufs=2, space="PSUM") as ps2_pool,
        ):
            b1_sb = const_pool.tile([P, NFT], f32, name="b1sb")
            b2_sb = const_pool.tile([P, NDT], f32, name="b2sb")
            nc.sync.dma_start(out=b1_sb[:], in_=b1[:])
            nc.sync.dma_start(out=b2_sb[:], in_=b2[:])

            for t in range(n_tch):
                xt_sb = xt_pool.tile([P, NDT, TCH], f32r, name="xts")
                nc.sync.dma_start(out=xt_sb[:], in_=xt[:, :, ds(t * TCH, TCH)])

                h_tiles = []
                for ft in range(NFT):
                    w1t = w1_pool.tile([P, NDT, P], f32r, name="w1t")
                    nc.sync.dma_start(out=w1t[:], in_=w1[:, ts(ft, 1)])
                    h_sb = h_pool.tile([P, TCH], f32r, name=f"h{ft}")
                    for sub in range(nsub):
                        sl = ds(sub * SUB, SUB)
                        ps1 = ps1_pool.tile([P, SUB], f32, name=f"ps1_{sub}")
                        for dt in range(NDT):
                            nc.tensor.matmul(
                                ps1[:],
                                w1t[:, ts(dt, 1)].squeeze(),
                                xt_sb[:, ts(dt, 1), sl].squeeze(),
                                start=(dt == 0),
                                stop=(dt == NDT - 1),
                            )
                        nc.scalar.activation(
                            h_sb[:, sl],
                            ps1[:],
                            mybir.ActivationFunctionType.Gelu,
                            bias=b1_sb[:, ts(ft, 1)],
                        )
                    h_tiles.append(h_sb)

                for dt2 in range(NDT):
                    w2t = w2_pool.tile([P, NFT, P], f32r, name="w2t")
                    nc.sync.dma_start(out=w2t[:], in_=w2[:, ts(dt2, 1)])
                    y_sb = y_pool.tile([P, TCH], f32, name="ysb")
                    for sub in range(nsub):
                        sl = ds(sub * SUB, SUB)
                        ps2 = ps2_pool.tile([P, SUB], f32, name=f"ps2_{sub}")
                        for ft in range(NFT):
                            nc.tensor.matmul(
                                ps2[:],
                                w2t[:, ts(ft, 1)].squeeze(),
                                h_tiles[ft][:, sl],
                                start=(ft == 0),
                                stop=(ft == NFT - 1),
                            )
                        nc.vector.tensor_scalar_add(
                            y_sb[:, sl], ps2[:], b2_sb[:, ts(dt2, 1)]
                        )
                    nc.sync.dma_start(
                        out=y[:, ts(dt2, 1), ds(t * TCH, TCH)], in_=y_sb[:]
                    )

    nc.compile()
    return nc


def _get_built(cap: int):
    if cap not in _BUILT:
        _BUILT[cap] = _build(cap)
    return _BUILT[cap]


def _route(x_flat, Wr, br):
    """Router: softmax over experts, top-2, renormalized. Pure numpy."""
    logits = x_flat.astype(np.float32) @ Wr.astype(np.float32) + br.astype(np.float32)
    m = logits.max(axis=-1, keepdims=True)
    p = np.exp(logits - m)
    p /= p.sum(axis=-1, keepdims=True)
    i0 = np.argmax(p, axis=-1)
    pm = p.copy()
    pm[np.arange(p.shape[0]), i0] = -np.inf
    i1 = np.argmax(pm, axis=-1)
    w0 = p[np.arange(p.shape[0]), i0]
    w1 = p[np.arange(p.shape[0]), i1]
    s = w0 + w1
    return i0, i1, w0 / s, w1 / s


def kernel(x, Wr, br, W1, b1, W2, b2, _run_kwargs=None):
    x = np.asarray(x)
    B, L, D = x.shape
    T = B * L
    x_flat = np.ascontiguousarray(x.reshape(T, D), dtype=np.float32)

    i0, i1, w0, w1c = _route(x_flat, Wr, br)

    rows_l, wts_l = [], []
    for e in range(NUM_EXPERTS):
        sel = (i0 == e) | (i1 == e)
        rows = np.nonzero(sel)[0]
        w = np.where(i0[rows] == e, w0[rows], w1c[rows]).astype(np.float32)
        rows_l.append(rows)
        wts_l.append(w)

    max_n = max(len(r) for r in rows_l)
    cap = CAP_DEFAULT
    while cap < max_n:
        cap += TCH
    nc = _get_built(cap)

    in_maps = []
    for e in range(NUM_EXPERTS):
        rows = rows_l[e]
        xe = np.zeros((cap, D_MODEL), dtype=np.float32)
        xe[: len(rows)] = x_flat[rows]
        # [cap, D] -> [D, cap] -> [NDT, P, cap] -> [P, NDT, cap]
        xtr = _tf32(
            np.ascontiguousarray(
                xe.T.reshape(NDT, P, cap).transpose(1, 0, 2)
            )
        )
        w1r = _tf32(
            np.ascontiguousarray(
                np.asarray(W1[e], dtype=np.float32)
                .reshape(NDT, P, NFT, P)
                .transpose(1, 2, 0, 3)
            )
        )
        w2r = _tf32(
            np.ascontiguousarray(
                np.asarray(W2[e], dtype=np.float32)
                .reshape(NFT, P, NDT, P)
                .transpose(1, 2, 0, 3)
            )
        )
        b1r = np.ascontiguousarray(
            np.asarray(b1[e], dtype=np.float32).reshape(NFT, P).T
        )
        b2r = np.ascontiguousarray(
            np.asarray(b2[e], dtype=np.float32).reshape(NDT, P).T
        )
        in_maps.append(
            {"xt": xtr, "w1": w1r, "w2": w2r, "b1": b1r, "b2": b2r}
        )

    kw = dict(_run_kwargs or {})
    res = run_bass_kernel_spmd(nc, in_maps, list(range(NUM_EXPERTS)), **kw)

    out = np.zeros((T, D_MODEL), dtype=np.float32)
    for e in range(NUM_EXPERTS):
        rows = rows_l[e]
        ye = np.asarray(res.results[e]["y"])  # [P, NDT, cap]
        ye = ye.transpose(1, 0, 2).reshape(D_MODEL, cap)  # [D, cap]
        out[rows] += wts_l[e][:, None] * ye[:, : len(rows)].T

    kernel._last_result = res
    kernel._last_in_maps = in_maps
    kernel._last_cap = cap
    return out.reshape(B, L, D_MODEL)


def make_bench_runner(nc, in_maps, n_cores=NUM_EXPERTS):
    """Device-resident repeat-execution runner for timing (mirrors
    bass2jax.run_bass_via_pjrt's multi-core path, but stages inputs on
    device once and creates donated zero outputs on-device)."""
    import jax
    import jax.numpy as jnp
    from jax.experimental.shard_map import shard_map
    from jax.sharding import Mesh, NamedSharding, PartitionSpec

    from concourse import bass2jax
    from concourse import mybir as _mybir

    bass2jax.install_neuronx_cc_hook()

    part_name = (
        nc.partition_id_tensor.name if nc.partition_id_tensor else None
    )
    in_names, out_names, out_avals = [], [], []
    for alloc in nc.m.functions[0].allocations:
        if not isinstance(alloc, _mybir.MemoryLocationSet):
            continue
        name = alloc.memorylocations[0].name
        if alloc.kind == "ExternalInput":
            if name != part_name:
                in_names.append(name)
        elif alloc.kind == "ExternalOutput":
            out_names.append(name)
            out_avals.append(
                jax.core.ShapedArray(
                    tuple(alloc.tensor_shape), _mybir.dt.np(alloc.dtype)
                )
            )
    n_params = len(in_names)
    all_in = in_names + out_names
    if part_name is not None:
        all_in = all_in + [part_name]

    def _body(*args):
        operands = list(args)
        if part_name is not None:
            operands.append(bass2jax.partition_id_tensor())
        outs = bass2jax._bass_exec_p.bind(
            *operands,
            out_avals=tuple(out_avals),
            in_names=tuple(all_in),
            out_names=tuple(out_names),
            lowering_input_output_aliases=(),
            sim_require_finite=True,
            sim_require_nnan=True,
            nc=nc,
        )
        return tuple(outs)

    devices = jax.devices()[:n_cores]
    mesh = Mesh(np.asarray(devices), ("core",))
    spec = NamedSharding(mesh, PartitionSpec("core"))
    donate = tuple(range(n_params, n_params + len(out_names)))
    sharded = jax.jit(
        shard_map(
            _body,
            mesh=mesh,
            in_specs=(PartitionSpec("core"),) * (n_params + len(out_names)),
            out_specs=(PartitionSpec("core"),) * len(out_names),
            check_rep=False,
        ),
        donate_argnums=donate,
        keep_unused=True,
    )
    din = [
        jax.device_put(
            np.concatenate([m[name] for m in in_maps], axis=0), spec
        )
        for name in in_names
    ]
    zero_shapes = [
        (n_cores * a.shape[0], *a.shape[1:]) for a in out_avals
    ]
    zeros_fn = jax.jit(
        lambda: tuple(
            jnp.zeros(s, a.dtype) for s, a in zip(zero_shapes, out_avals)
        ),
        out_shardings=tuple(spec for _ in out_avals),
    )

    def run_once():
        return sharded(*din, *zeros_fn())

    def zeros_only():
        return zeros_fn()

    return run_once, zeros_only


# revision 14
# speedup vs baseline: 5.0828x; 5.0828x over previous
"""MoE (8 experts, top-2) Trainium2 kernel.

Strategy (expert-parallel, per sharding hint):
  - Host: router (softmax + top-2 + renorm)  [0.1% of FLOPs], then
    all-to-all dispatch done host-side: gather each expert's tokens into a
    fixed-capacity buffer.
  - Device (8 cores, 1 expert each): Y_e = GELU(X_e @ W1[e] + b1[e]) @ W2[e] + b2[e]
    Matmuls run in fp32r (tf32) on the PE array: full 1-cycle/row rate.
  - Host: weighted combine (the return all-to-all) back to [B, L, D].

Device loop per core (capacity C tokens, chunks of 768):
  mm1: psum1[f128, tok] = sum_dt W1t[dt,ft].T @ Xt[dt, tok]   (f32r)
  h[ft] = Gelu(psum1 + b1[ft])  -> SBUF f32r (resident for whole chunk)
  mm2: psum2[d128, tok] = sum_ft W2t[ft,dt2].T @ h[ft, tok]   (f32r)
  y = psum2 + b2 -> DRAM
"""

import numpy as np

import concourse.bacc as bacc
import concourse.bass_utils as _bu
import concourse.mybir as mybir
import concourse.tile as tile
from concourse.bass import ds, ts
from concourse.bass_utils import run_bass_kernel_spmd

# Enable walrus's LDWEIGHTS optimization for NEFFs compiled by this module:
# the fused fp32r matmuls reload the stationary tile every instruction and
# ldw-opt overlaps/dedupes those loads (measured 665us -> 583us per kernel
# iteration, bit-identical outputs).
if not getattr(_bu.run_command, "_ldw_opt_patched", False):
    _orig_run_command = _bu.run_command

    def _run_command_ldw_opt(cmd, **kw):
        if isinstance(cmd, list):
            cmd = [
                "--enable-ldw-opt=true" if c == "--enable-ldw-opt=false" else c
                for c in cmd
            ]
        return _orig_run_command(cmd, **kw)

    _run_command_ldw_opt._ldw_opt_patched = True
    _bu.run_command = _run_command_ldw_opt

P = 128
D_MODEL = 1024
D_FF = 4096
NUM_EXPERTS = 8
TOP_K = 2
NDT = D_MODEL // P   # 8  d-tiles
NFT = D_FF // P      # 32 f-tiles
CAP_DEFAULT = 2304   # tokens-per-expert capacity (multiple of TCH)
TCH = 768            # token chunk resident in SBUF
SUB = 384            # matmul moving free dim (>=256 keeps fp32r at full rate)

f32 = mybir.dt.float32
f32r = mybir.dt.float32r

_BUILT = {}


def _tf32(a: np.ndarray) -> np.ndarray:
    """Round-to-nearest-even fp32 -> tf32 (19 low mantissa bits -> 10)."""
    u = np.ascontiguousarray(a, dtype=np.float32).view(np.uint32)
    u = (u + np.uint32(0xFFF) + ((u >> np.uint32(13)) & np.uint32(1))) & np.uint32(
        0xFFFFE000
    )
    return u.view(np.float32)


def _build(cap: int, repeats: int = 1):
    """Build the per-core expert-MLP Bass module for a given capacity.

    repeats>1 re-runs the whole pipeline (for slope-based HW timing)."""
    nc = bacc.Bacc(None, target_bir_lowering=False)

    xt = nc.declare_dram_parameter("xt", [P, NDT, cap], f32r, isOutput=False)
    w1 = nc.declare_dram_parameter("w1", [P, NFT, NDT, P], f32r, isOutput=False)
    w2 = nc.declare_dram_parameter("w2", [P, NDT, NFT, P], f32r, isOutput=False)
    b1 = nc.declare_dram_parameter("b1", [P, NFT], f32, isOutput=False)
    b2 = nc.declare_dram_parameter("b2", [P, NDT], f32, isOutput=False)
    y = nc.declare_dram_parameter("y", [P, NDT, cap], f32, isOutput=True)

    n_tch = cap // TCH
    nsub = TCH // SUB

    with tile.TileContext(nc) as tc:
        with (
            tc.tile_pool(name="const", bufs=1) as const_pool,
            tc.tile_pool(name="xt", bufs=1) as xt_pool,
            tc.tile_pool(name="w1", bufs=2) as w1_pool,
            tc.tile_pool(name="w2", bufs=2) as w2_pool,
            tc.tile_pool(name="h", bufs=1) as h_pool,
            tc.tile_pool(name="yo", bufs=2) as y_pool,
            tc.tile_pool(name="ps1", bufs=2, space="PSUM") as ps1_pool,
            tc.tile_pool(name="ps2", bufs=2, space="PSUM") as ps2_pool,
        ):
            b1_sb = const_pool.tile([P, NFT], f32, name="b1sb")
            b2_sb = const_pool.tile([P, NDT], f32, name="b2sb")
            nc.sync.dma_start(out=b1_sb[:], in_=b1[:])
            nc.sync.dma_start(out=b2_sb[:], in_=b2[:])

            for t in [tt % n_tch for tt in range(n_tch * repeats)]:
                xt_sb = xt_pool.tile([P, NDT, TCH], f32r, name="xts")
                nc.sync.dma_start(out=xt_sb[:], in_=xt[:, :, ds(t * TCH, TCH)])

                h_tiles = []
                for ft in range(NFT):
                    w1t = w1_pool.tile([P, NDT, P], f32r, name="w1t")
                    nc.sync.dma_start(out=w1t[:], in_=w1[:, ts(ft, 1)])
                    h_sb = h_pool.tile([P, TCH], f32r, name=f"h{ft}")
                    ps1 = [
                        ps1_pool.tile([P, SUB], f32, name=f"ps1_{sub}")
                        for sub in range(nsub)
                    ]
                    # dt outer / sub inner: stationary W1 tile loaded once
                    # per dt instead of once per (sub, dt).
                    for dt in range(NDT):
                        for sub in range(nsub):
                            sl = ds(sub * SUB, SUB)
                            nc.tensor.matmul(
                                ps1[sub][:],
                                w1t[:, ts(dt, 1)].squeeze(),
                                xt_sb[:, ts(dt, 1), sl].squeeze(),
                                start=(dt == 0),
                                stop=(dt == NDT - 1),
                                skip_group_check=True,
                            )
                    for sub in range(nsub):
                        nc.scalar.activation(
                            h_sb[:, ds(sub * SUB, SUB)],
                            ps1[sub][:],
                            mybir.ActivationFunctionType.Gelu,
                            bias=b1_sb[:, ts(ft, 1)],
                        )
                    h_tiles.append(h_sb)

                for dt2 in range(NDT):
                    w2t = w2_pool.tile([P, NFT, P], f32r, name="w2t")
                    nc.sync.dma_start(out=w2t[:], in_=w2[:, ts(dt2, 1)])
                    y_sb = y_pool.tile([P, TCH], f32, name="ysb")
                    ps2 = [
                        ps2_pool.tile([P, SUB], f32, name=f"ps2_{sub}")
                        for sub in range(nsub)
                    ]
                    # ft outer / sub inner: W2 tile loaded once per ft.
                    for ft in range(NFT):
                        for sub in range(nsub):
                            nc.tensor.matmul(
                                ps2[sub][:],
                                w2t[:, ts(ft, 1)].squeeze(),
                                h_tiles[ft][:, ds(sub * SUB, SUB)],
                                start=(ft == 0),
                                stop=(ft == NFT - 1),
                                skip_group_check=True,
                            )
                    for sub in range(nsub):
                        nc.vector.tensor_scalar_add(
                            y_sb[:, ds(sub * SUB, SUB)],
                            ps2[sub][:],
                            b2_sb[:, ts(dt2, 1)],
                        )
                    nc.sync.dma_start(
                        out=y[:, ts(dt2, 1), ds(t * TCH, TCH)], in_=y_sb[:]
                    )

    nc.compile()
    return nc


def _get_built(cap: int, repeats: int = 1):
    key = (cap, repeats)
    if key not in _BUILT:
        _BUILT[key] = _build(cap, repeats)
    return _BUILT[key]


def _route(x_flat, Wr, br):
    """Router: softmax over experts, top-2, renormalized. Pure numpy."""
    logits = x_flat.astype(np.float32) @ Wr.astype(np.float32) + br.astype(np.float32)
    m = logits.max(axis=-1, keepdims=True)
    p = np.exp(logits - m)
    p /= p.sum(axis=-1, keepdims=True)
    i0 = np.argmax(p, axis=-1)
    pm = p.copy()
    pm[np.arange(p.shape[0]), i0] = -np.inf
    i1 = np.argmax(pm, axis=-1)
    w0 = p[np.arange(p.shape[0]), i0]
    w1 = p[np.arange(p.shape[0]), i1]
    s = w0 + w1
    return i0, i1, w0 / s, w1 / s


def kernel(x, Wr, br, W1, b1, W2, b2, _run_kwargs=None):
    x = np.asarray(x)
    B, L, D = x.shape
    T = B * L
    x_flat = np.ascontiguousarray(x.reshape(T, D), dtype=np.float32)

    i0, i1, w0, w1c = _route(x_flat, Wr, br)

    rows_l, wts_l = [], []
    for e in range(NUM_EXPERTS):
        sel = (i0 == e) | (i1 == e)
        rows = np.nonzero(sel)[0]
        w = np.where(i0[rows] == e, w0[rows], w1c[rows]).astype(np.float32)
        rows_l.append(rows)
        wts_l.append(w)

    max_n = max(len(r) for r in rows_l)
    cap = CAP_DEFAULT
    while cap < max_n:
        cap += TCH
    nc = _get_built(cap)

    in_maps = []
    for e in range(NUM_EXPERTS):
        rows = rows_l[e]
        xe = np.zeros((cap, D_MODEL), dtype=np.float32)
        xe[: len(rows)] = x_flat[rows]
        # [cap, D] -> [D, cap] -> [NDT, P, cap] -> [P, NDT, cap]
        xtr = _tf32(
            np.ascontiguousarray(
                xe.T.reshape(NDT, P, cap).transpose(1, 0, 2)
            )
        )
        w1r = _tf32(
            np.ascontiguousarray(
                np.asarray(W1[e], dtype=np.float32)
                .reshape(NDT, P, NFT, P)
                .transpose(1, 2, 0, 3)
            )
        )
        w2r = _tf32(
            np.ascontiguousarray(
                np.asarray(W2[e], dtype=np.float32)
                .reshape(NFT, P, NDT, P)
                .transpose(1, 2, 0, 3)
            )
        )
        b1r = np.ascontiguousarray(
            np.asarray(b1[e], dtype=np.float32).reshape(NFT, P).T
        )
        b2r = np.ascontiguousarray(
            np.asarray(b2[e], dtype=np.float32).reshape(NDT, P).T
        )
        in_maps.append(
            {"xt": xtr, "w1": w1r, "w2": w2r, "b1": b1r, "b2": b2r}
        )

    kw = dict(_run_kwargs or {})
    res = run_bass_kernel_spmd(nc, in_maps, list(range(NUM_EXPERTS)), **kw)

    out = np.zeros((T, D_MODEL), dtype=np.float32)
    for e in range(NUM_EXPERTS):
        rows = rows_l[e]
        ye = np.asarray(res.results[e]["y"])  # [P, NDT, cap]
        ye = ye.transpose(1, 0, 2).reshape(D_MODEL, cap)  # [D, cap]
        out[rows] += wts_l[e][:, None] * ye[:, : len(rows)].T

    kernel._last_result = res
    kernel._last_in_maps = in_maps
    kernel._last_cap = cap
    return out.reshape(B, L, D_MODEL)


def make_bench_runner(nc, in_maps, n_cores=NUM_EXPERTS):
    """Device-resident repeat-execution runner for timing (mirrors
    bass2jax.run_bass_via_pjrt's multi-core path, but stages inputs on
    device once and creates donated zero outputs on-device)."""
    import jax
    import jax.numpy as jnp
    from jax.experimental.shard_map import shard_map
    from jax.sharding import Mesh, NamedSharding, PartitionSpec

    from concourse import bass2jax
    from concourse import mybir as _mybir

    bass2jax.install_neuronx_cc_hook()

    part_name = (
        nc.partition_id_tensor.name if nc.partition_id_tensor else None
    )
    in_names, out_names, out_avals = [], [], []
    for alloc in nc.m.functions[0].allocations:
        if not isinstance(alloc, _mybir.MemoryLocationSet):
            continue
        name = alloc.memorylocations[0].name
        if alloc.kind == "ExternalInput":
            if name != part_name:
                in_names.append(name)
        elif alloc.kind == "ExternalOutput":
            out_names.append(name)
            out_avals.append(
                jax.core.ShapedArray(
                    tuple(alloc.tensor_shape), _mybir.dt.np(alloc.dtype)
                )
            )
    n_params = len(in_names)
    all_in = in_names + out_names
    if part_name is not None:
        all_in = all_in + [part_name]

    def _body(*args):
        operands = list(args)
        if part_name is not None:
            operands.append(bass2jax.partition_id_tensor())
        outs = bass2jax._bass_exec_p.bind(
            *operands,
            out_avals=tuple(out_avals),
            in_names=tuple(all_in),
            out_names=tuple(out_names),
            lowering_input_output_aliases=(),
            sim_require_finite=True,
            sim_require_nnan=True,
            nc=nc,
        )
        return tuple(outs)

    devices = jax.devices()[:n_cores]
    mesh = Mesh(np.asarray(devices), ("core",))
    spec = NamedSharding(mesh, PartitionSpec("core"))
    donate = tuple(range(n_params, n_params + len(out_names)))
    sharded = jax.jit(
        shard_map(
            _body,
            mesh=mesh,
            in_specs=(PartitionSpec("core"),) * (n_params + len(out_names)),
            out_specs=(PartitionSpec("core"),) * len(out_names),
            check_rep=False,
        ),
        donate_argnums=donate,
        keep_unused=True,
    )
    din = [
        jax.device_put(
            np.concatenate([m[name] for m in in_maps], axis=0), spec
        )
        for name in in_names
    ]
    zero_shapes = [
        (n_cores * a.shape[0], *a.shape[1:]) for a in out_avals
    ]
    zeros_fn = jax.jit(
        lambda: tuple(
            jnp.zeros(s, a.dtype) for s, a in zip(zero_shapes, out_avals)
        ),
        out_shardings=tuple(spec for _ in out_avals),
    )

    def run_once():
        return sharded(*din, *zeros_fn())

    def zeros_only():
        return zeros_fn()

    return run_once, zeros_only
